# revision 14
# baseline (speedup 1.0000x reference)
"""Conv1d (B=32, C_in=256, L=4096, C_out=512, K=9, stride=1, pad=4) on 8 trn2 cores.

Data-parallel over batch: 4 batches per core; weights/bias broadcast.
Per core: out[b, t, co] = sum_{ci,k} x_pad[b, ci, t+k] * w[co, ci, k] + bias[co]
computed as 18 PSUM-accumulated matmuls per 128-position output tile:
  stationary lhsT = x_pad[ci(128), t(128)]  (slid by k)
  moving    rhs  = w_k[ci(128), co(512)]    (host-pre-transposed to [K, C_in, C_out])
PSUM tile [t(128), co(512)] -> +bias on DVE -> DMA to (B, T, C_out) output.
"""

import numpy as np

B, C_IN, L = 32, 256, 4096
C_OUT, KW = 512, 9
PAD = 4
N_CORES = 8
B_LOC = B // N_CORES  # 4
P = 128
CI_CHUNKS = C_IN // P  # 2
T_TILE = 128
LP = L + 2 * PAD  # 4104
N_TT = L // T_TILE  # 32

# matmul input dtype mode: "f32r" (full-rate), "f32" (exact, 4x slower)
MM_MODE = "f32r"

_cache = {}


def _build_program(repeat=1):
    from contextlib import ExitStack

    import concourse.tile as tile
    from concourse import bacc, mybir

    f32 = mybir.dt.float32
    mm_dt = mybir.dt.float32r if MM_MODE == "f32r" else mybir.dt.float32

    nc = bacc.Bacc("TRN2", debug=False)
    x_d = nc.dram_tensor("x", [B_LOC, C_IN, LP], mm_dt, kind="ExternalInput").ap()
    w_d = nc.dram_tensor("w", [KW, C_IN, C_OUT], mm_dt, kind="ExternalInput").ap()
    b_d = nc.dram_tensor("bias", [C_OUT], f32, kind="ExternalInput").ap()
    o_d = nc.dram_tensor("out", [B_LOC, L, C_OUT], f32, kind="ExternalOutput").ap()

    with tile.TileContext(nc) as tc:
        with ExitStack() as ctx:
            persist = ctx.enter_context(tc.tile_pool(name="persist", bufs=1))
            wt = persist.tile(
                [P, KW * CI_CHUNKS * C_OUT], mm_dt, name="wt", tag="wt"
            )
            bias_sb = persist.tile([P, C_OUT], f32, name="bias_sb", tag="bias")
            xps = [
                persist.tile([P, CI_CHUNKS * LP], mm_dt, name=f"xp{i}", tag=f"xp{i}")
                for i in range(2)
            ]

            psum_pool = ctx.enter_context(
                tc.tile_pool(name="psum", bufs=8, space="PSUM")
            )
            out_pool = ctx.enter_context(tc.tile_pool(name="outs", bufs=6))

            # Weights: wt column block (k*2+c) holds w[k, c*128:(c+1)*128, :].
            for k in range(KW):
                for c in range(CI_CHUNKS):
                    j = (k * CI_CHUNKS + c) * C_OUT
                    nc.sync.dma_start(
                        out=wt[:, j : j + C_OUT], in_=w_d[k, c * P : (c + 1) * P, :]
                    )
            nc.sync.dma_start(
                out=bias_sb[:], in_=b_d.unsqueeze(0).to_broadcast((P, C_OUT))
            )

            def body():
                for b in range(B_LOC):
                    xp = xps[b % 2]
                    for c in range(CI_CHUNKS):
                        nc.sync.dma_start(
                            out=xp[:, c * LP : (c + 1) * LP],
                            in_=x_d[b, c * P : (c + 1) * P, :],
                        )
                    for ti in range(N_TT):
                        t0 = ti * T_TILE
                        ps = psum_pool.tile([P, C_OUT], f32, name="ps")
                        n_mm = KW * CI_CHUNKS
                        i = 0
                        for c in range(CI_CHUNKS):
                            for k in range(KW):
                                j = (k * CI_CHUNKS + c) * C_OUT
                                nc.tensor.matmul(
                                    ps[:],
                                    lhsT=xp[
                                        :, c * LP + t0 + k : c * LP + t0 + k + T_TILE
                                    ],
                                    rhs=wt[:, j : j + C_OUT],
                                    start=(i == 0),
                                    stop=(i == n_mm - 1),
                                )
                                i += 1
                        ob = out_pool.tile([P, C_OUT], f32, name="ob")
                        nc.vector.tensor_add(ob[:], ps[:], bias_sb[:])
                        nc.sync.dma_start(
                            out=o_d[b, t0 : t0 + T_TILE, :], in_=ob[:]
                        )

            for _ in range(repeat):
                body()

    nc.compile()
    return nc


def _get_program(repeat=1):
    key = ("nc", repeat)
    if key not in _cache:
        _cache[key] = _build_program(repeat)
    return _cache[key]


def _make_in_maps(x, w, bias):
    wt = np.ascontiguousarray(np.transpose(w, (2, 1, 0)))  # (K, C_in, C_out)
    xp = np.pad(x, ((0, 0), (0, 0), (PAD, PAD)))  # (B, C_in, L+2*PAD)
    return [
        {
            "x": np.ascontiguousarray(xp[c * B_LOC : (c + 1) * B_LOC]),
            "w": wt,
            "bias": bias,
        }
        for c in range(N_CORES)
    ]


def kernel(**inputs):
    from concourse.bass_utils import run_bass_kernel_spmd

    x = np.ascontiguousarray(inputs["x"], dtype=np.float32)
    w = np.ascontiguousarray(inputs["weight"], dtype=np.float32)
    bias = np.ascontiguousarray(inputs["bias"], dtype=np.float32)

    nc = _get_program()
    res = run_bass_kernel_spmd(nc, _make_in_maps(x, w, bias), list(range(N_CORES)))
    return np.concatenate(
        [res.results[c]["out"] for c in range(N_CORES)], axis=0
    )


# revision 18
# speedup vs baseline: 6.4551x; 6.4551x over previous
"""Conv1d (B=32, C_in=256, L=4096, C_out=512, K=9, stride=1, pad=4) on 8 trn2 cores.

Data-parallel over batch: 4 batches per core; weights/bias broadcast.
Per core: out[b, t, co] = sum_{ci,k} x_pad[b, ci, t+k] * w[co, ci, k] + bias[co]
computed as 18 PSUM-accumulated matmuls per 128-position output tile:
  stationary lhsT = x_pad[ci(128), t(128)]  (slid by k)
  moving    rhs  = w_k[ci(128), co(512)]    (host-pre-transposed to [K, C_in, C_out])
PSUM tile [t(128), co(512)] -> +bias on DVE -> DMA to (B, T, C_out) output.
"""

import numpy as np

B, C_IN, L = 32, 256, 4096
C_OUT, KW = 512, 9
PAD = 4
N_CORES = 8
B_LOC = B // N_CORES  # 4
P = 128
CI_CHUNKS = C_IN // P  # 2
T_TILE = 128
LP = L + 2 * PAD  # 4104
N_TT = L // T_TILE  # 32

# matmul input dtype mode: "f32r" (full-rate), "f32" (exact, 4x slower)
MM_MODE = "f32r"

_cache = {}


def _build_program(repeat=1):
    from contextlib import ExitStack

    import concourse.tile as tile
    from concourse import bacc, mybir

    f32 = mybir.dt.float32
    mm_dt = mybir.dt.float32r if MM_MODE == "f32r" else mybir.dt.float32

    nc = bacc.Bacc("TRN2", debug=False)
    x_d = nc.dram_tensor("x", [B_LOC, C_IN, LP], mm_dt, kind="ExternalInput").ap()
    w_d = nc.dram_tensor("w", [KW, C_IN, C_OUT], mm_dt, kind="ExternalInput").ap()
    b_d = nc.dram_tensor("bias", [C_OUT], f32, kind="ExternalInput").ap()
    o_d = nc.dram_tensor("out", [B_LOC, L, C_OUT], f32, kind="ExternalOutput").ap()

    with tile.TileContext(nc) as tc:
        with ExitStack() as ctx:
            persist = ctx.enter_context(tc.tile_pool(name="persist", bufs=1))
            wt = persist.tile(
                [P, KW * CI_CHUNKS * C_OUT], mm_dt, name="wt", tag="wt"
            )
            bias_sb = persist.tile([P, C_OUT], f32, name="bias_sb", tag="bias")
            xps = [
                persist.tile([P, CI_CHUNKS * LP], mm_dt, name=f"xp{i}", tag=f"xp{i}")
                for i in range(2)
            ]

            psum_pool = ctx.enter_context(
                tc.tile_pool(name="psum", bufs=8, space="PSUM")
            )
            out_pool = ctx.enter_context(tc.tile_pool(name="outs", bufs=6))

            NS = 8  # x DMA slices per (batch, ci-chunk): finer deps, earlier start
            SW = LP // NS  # 513
            assert SW * NS == LP

            def emit_w(k):
                # wt column block (k*2+c) holds w[k, c*128:(c+1)*128, :].
                for c in range(CI_CHUNKS):
                    j = (k * CI_CHUNKS + c) * C_OUT
                    nc.sync.dma_start(
                        out=wt[:, j : j + C_OUT], in_=w_d[k, c * P : (c + 1) * P, :]
                    )

            def emit_x(b, slices=range(NS)):
                xp = xps[b % 2]
                for s in slices:
                    for c in range(CI_CHUNKS):
                        nc.sync.dma_start(
                            out=xp[:, c * LP + s * SW : c * LP + (s + 1) * SW],
                            in_=x_d[b, c * P : (c + 1) * P, s * SW : (s + 1) * SW],
                        )

            # Emission order shapes DMA priority: first-needed data first —
            # k=0 weights, x slice 0, remaining weights, remaining x slices.
            emit_w(0)
            emit_x(0, slices=[0])
            for k in range(1, KW):
                emit_w(k)
            nc.sync.dma_start(
                out=bias_sb[:], in_=b_d.unsqueeze(0).to_broadcast((P, C_OUT))
            )
            emit_x(0, slices=range(1, NS))

            def body(first=False):
                for b in range(B_LOC):
                    if not (first and b == 0):
                        emit_x(b)
                    xp = xps[b % 2]
                    for ti in range(N_TT):
                        t0 = ti * T_TILE
                        ps = psum_pool.tile([P, C_OUT], f32, name="ps")
                        n_mm = KW * CI_CHUNKS
                        i = 0
                        for k in range(KW):
                            for c in range(CI_CHUNKS):
                                j = (k * CI_CHUNKS + c) * C_OUT
                                nc.tensor.matmul(
                                    ps[:],
                                    lhsT=xp[
                                        :, c * LP + t0 + k : c * LP + t0 + k + T_TILE
                                    ],
                                    rhs=wt[:, j : j + C_OUT],
                                    start=(i == 0),
                                    stop=(i == n_mm - 1),
                                )
                                i += 1
                        ob = out_pool.tile([P, C_OUT], f32, name="ob")
                        nc.vector.tensor_add(ob[:], ps[:], bias_sb[:])
                        nc.sync.dma_start(
                            out=o_d[b, t0 : t0 + T_TILE, :], in_=ob[:]
                        )

            for r in range(repeat):
                body(first=(r == 0))

    nc.compile()
    return nc


def _get_program(repeat=1):
    key = ("nc", repeat)
    if key not in _cache:
        _cache[key] = _build_program(repeat)
    return _cache[key]


def _make_in_maps(x, w, bias):
    wt = np.ascontiguousarray(np.transpose(w, (2, 1, 0)))  # (K, C_in, C_out)
    xp = np.pad(x, ((0, 0), (0, 0), (PAD, PAD)))  # (B, C_in, L+2*PAD)
    return [
        {
            "x": np.ascontiguousarray(xp[c * B_LOC : (c + 1) * B_LOC]),
            "w": wt,
            "bias": bias,
        }
        for c in range(N_CORES)
    ]


def kernel(**inputs):
    from concourse.bass_utils import run_bass_kernel_spmd

    x = np.ascontiguousarray(inputs["x"], dtype=np.float32)
    w = np.ascontiguousarray(inputs["weight"], dtype=np.float32)
    bias = np.ascontiguousarray(inputs["bias"], dtype=np.float32)

    nc = _get_program()
    res = run_bass_kernel_spmd(nc, _make_in_maps(x, w, bias), list(range(N_CORES)))
    return np.concatenate(
        [res.results[c]["out"] for c in range(N_CORES)], axis=0
    )
